# revision 3
# baseline (speedup 1.0000x reference)
"""Multi-head attention (B=4, S=1024, H=1024, heads=16) on 8 trn2 NeuronCores.

Sharding: data-parallel over batch (4) x tensor-parallel over head-groups (2).
Core c handles batch c//2, heads [8*(c%2), 8*(c%2)+8).

v2 schedule: the tensor queue is kept dense by interleaving projection
matmul bursts into the previous head-pair's attention loop, scores matmuls
alternate PE row-groups (hl0 rows 0-63 / hl1 rows 64-127) so they run
concurrently via tile_position, and the output projection accumulates
pr0-2 before pr3 so pr3's softmax-normalize hides under real matmuls.

Per-core math (all matmuls bf16 with fp32 PSUM accumulation):
  - projections: qh_T/kh_T in [d, i] layout (head dim on partitions), vh in
    [j, hd] layout augmented with a ones column per head (softmax
    denominator comes for free from the ctx matmul).
  - scores computed transposed (keys on partitions): s_T = khT^T @ qhT,
    exp on ScalarE, multiplied by host-precomputed exp(attn_bias)^T.
  - normalize with reciprocal of the denominator row + DMA partition
    broadcast, output projection with row-parallel Wo; host adds the two
    partial results + bo.

Scale (1/8) is folded into Wq/bq on the host. Softmax max-subtraction is
skipped: scores+bias are within +-8 so exp is well-conditioned in fp32.
"""

import numpy as np
import ml_dtypes

BF16 = ml_dtypes.bfloat16

S = 1024
HID = 1024
GCOL = 512  # hidden cols per core (8 heads * 64)
DH = 64
P = 128
NPAIR = 4  # head pairs per core
NJB = 8  # key blocks of 128
NCB = 8  # contraction blocks of 128
NIB = 8  # query blocks of 128

_CACHED_NC = None


def _build_nc():
    import concourse.bass as bass
    import concourse.mybir as mybir
    import concourse.tile as tile
    from concourse import bacc
    from contextlib import ExitStack

    f32 = mybir.dt.float32
    bf16 = mybir.dt.bfloat16
    AF = mybir.ActivationFunctionType

    nc = bacc.Bacc(
        "TRN2",
        target_bir_lowering=False,
        debug=False,
        enable_asserts=False,
        num_devices=8,
    )

    qT = nc.dram_tensor("qT", [HID, S], bf16, kind="ExternalInput").ap()
    kT = nc.dram_tensor("kT", [HID, S], bf16, kind="ExternalInput").ap()
    vT = nc.dram_tensor("vT", [HID, S], bf16, kind="ExternalInput").ap()
    wq = nc.dram_tensor("wq", [HID, GCOL], bf16, kind="ExternalInput").ap()
    wk = nc.dram_tensor("wk", [HID, GCOL], bf16, kind="ExternalInput").ap()
    wv = nc.dram_tensor("wv", [HID, GCOL], bf16, kind="ExternalInput").ap()
    wo = nc.dram_tensor("wo", [GCOL, HID], bf16, kind="ExternalInput").ap()
    bq = nc.dram_tensor("bq", [GCOL], f32, kind="ExternalInput").ap()
    bk = nc.dram_tensor("bk", [GCOL], f32, kind="ExternalInput").ap()
    bv = nc.dram_tensor("bv", [GCOL], bf16, kind="ExternalInput").ap()
    expb = nc.dram_tensor("expb", [8, S, S], bf16, kind="ExternalInput").ap()
    out = nc.dram_tensor("out", [S, HID], f32, kind="ExternalOutput").ap()

    with tile.TileContext(nc) as tc, ExitStack() as ctx:
        const = ctx.enter_context(tc.tile_pool(name="const", bufs=1))
        inT = ctx.enter_context(tc.tile_pool(name="inT", bufs=24))
        proj = ctx.enter_context(tc.tile_pool(name="proj", bufs=1))
        work = ctx.enter_context(tc.tile_pool(name="work", bufs=6))
        outp = ctx.enter_context(tc.tile_pool(name="outp", bufs=2))
        psum = ctx.enter_context(tc.tile_pool(name="psum", bufs=2, space="PSUM"))

        # ---- constants / weights (DMA priority: q, k first — scores path) ----
        wq_sb = const.tile([P, NCB, GCOL], bf16, tag="wq")
        wk_sb = const.tile([P, NCB, GCOL], bf16, tag="wk")
        wv_sb = const.tile([P, NCB, GCOL], bf16, tag="wv")
        wo_sb = const.tile([P, NPAIR, HID], bf16, tag="wo")
        wq_r = wq.rearrange("(cb p) n -> p cb n", p=P)
        wk_r = wk.rearrange("(cb p) n -> p cb n", p=P)
        wv_r = wv.rearrange("(cb p) n -> p cb n", p=P)
        bq_sb = const.tile([P, NPAIR], f32, tag="bq")
        bk_sb = const.tile([P, NPAIR], f32, tag="bk")
        bv_sb = const.tile([1, GCOL], bf16, tag="bv")
        ones_k1 = const.tile([1, P], bf16, tag="ones_k1")
        nc.vector.memset(ones_k1, 1.0)
        # pre-warm the Exp activation table before the attention phase
        warm = const.tile([1, 16], bf16, tag="warm")
        nc.vector.memset(warm, 0.0)
        nc.scalar.activation(warm, warm, AF.Exp)

        qhT = [proj.tile([P, S], bf16, name=f"qhT{i}", tag=f"qhT{i}") for i in range(NPAIR)]
        khT = [proj.tile([P, S], bf16, name=f"khT{i}", tag=f"khT{i}") for i in range(NPAIR)]
        # vh_sb[jb]: [j in block, head, 65] where col 64 is ones (denominator trick)
        vh_sb = [proj.tile([P, 8, DH + 1], bf16, name=f"vh{i}", tag=f"vh{i}") for i in range(NJB)]
        ctxn = [proj.tile([P, S], bf16, name=f"ctxn{i}", tag=f"ctxn{i}") for i in range(NPAIR)]

        # input DMAs: q/k inputs + weights first so pr0 scores start early
        qk_tiles = {}
        nc.sync.dma_start(out=bq_sb, in_=bq.rearrange("(pr p) -> p pr", p=P))
        nc.sync.dma_start(out=bk_sb, in_=bk.rearrange("(pr p) -> p pr", p=P))
        for tname, src, w_r, w_sb in (("q", qT, wq_r, wq_sb), ("k", kT, wk_r, wk_sb)):
            tl = []
            for cb in range(NCB):
                nc.sync.dma_start(out=w_sb[:, cb, :], in_=w_r[:, cb, :])
                t = inT.tile([P, S], bf16, name=f"{tname}T{cb}", tag="inT")
                nc.sync.dma_start(out=t, in_=src[cb * P:(cb + 1) * P, :])
                tl.append(t)
            qk_tiles[tname] = tl
        vtiles = []
        nc.sync.dma_start(out=bv_sb, in_=bv.rearrange("(a n) -> a n", a=1))
        for cb in range(NCB):
            nc.sync.dma_start(out=wv_sb[:, cb, :], in_=wv_r[:, cb, :])
            t = inT.tile([P, S], bf16, name=f"vT{cb}", tag="inT")
            nc.sync.dma_start(out=t, in_=vT[cb * P:(cb + 1) * P, :])
            vtiles.append(t)

        def qk_half_burst(pr, tname, ic):
            """8 matmuls accumulating one [128, 512] half of q/k projection."""
            w_sb, b_sb, dst = (
                (wq_sb, bq_sb, qhT) if tname == "q" else (wk_sb, bk_sb, khT)
            )
            ps = psum.tile([P, S], f32, name=f"{tname}p{pr}_{ic}", tag="mm")
            for cb in range(NCB):
                nc.tensor.matmul(
                    ps[:, 0:512],
                    lhsT=w_sb[:, cb, pr * P:(pr + 1) * P],
                    rhs=qk_tiles[tname][cb][:, ic * 512:(ic + 1) * 512],
                    start=(cb == 0),
                    stop=(cb == NCB - 1),
                )
            nc.vector.tensor_scalar_add(
                dst[pr][:, ic * 512:(ic + 1) * 512], ps[:, 0:512], b_sb[:, pr:pr + 1]
            )

        def v_burst(jb):
            ps = psum.tile([P, S], f32, name=f"vp{jb}", tag="mm")
            for cb in range(NCB):
                nc.tensor.matmul(
                    ps[:, 0:GCOL],
                    lhsT=vtiles[cb][:, jb * P:(jb + 1) * P],
                    rhs=wv_sb[:, cb, :],
                    start=(cb == 0),
                    stop=False,
                )
            nc.tensor.matmul(ps[:, 0:GCOL], lhsT=ones_k1, rhs=bv_sb, start=False, stop=True)
            nc.vector.tensor_copy(
                out=vh_sb[jb][:, :, 0:DH],
                in_=ps[:, 0:GCOL].rearrange("p (h d) -> p h d", d=DH),
            )
            nc.vector.memset(vh_sb[jb][:, :, DH:DH + 1], 1.0)

        def normalize_head(pr, hl, cr, dma=None):
            """Normalize ctx rows by 1/r and place into ctxn[pr]."""
            dma = dma or nc.gpsimd
            # stage raw r (psum row 64) to sbuf (lane-aligned at p64), shift to
            # partition 0 by DMA, invert the single row, then DMA-broadcast the
            # inverse to partitions 0..63 and multiply.
            rb = work.tile([P, S], f32, name=f"rb{pr}{hl}", tag="rb", bufs=2)
            nc.vector.tensor_copy(rb[DH:DH + 1, :], cr[hl][DH:DH + 1, :])
            dma.dma_start(out=rb[0:1, :], in_=rb[DH:DH + 1, :])
            nc.vector.reciprocal(rb[0:1, :], rb[0:1, :])
            row = rb[0:1, :]
            row8 = bass.AP(
                tensor=row.tensor,
                offset=row.offset,
                ap=[list(row.ap[0]), [0, 7]] + [list(d) for d in row.ap[1:]],
            )
            dma.dma_start(out=rb[1:8, :], in_=row8)
            blk = rb[0:8, :]
            blk_rep = bass.AP(
                tensor=blk.tensor,
                offset=blk.offset,
                ap=[list(blk.ap[0]), [0, 7]] + [list(d) for d in blk.ap[1:]],
            )
            dma.dma_start(out=rb[8:DH, :], in_=blk_rep)
            if hl == 0:
                nc.vector.tensor_mul(ctxn[pr][0:DH, :], cr[hl][0:DH, :], rb[0:DH, :])
            else:
                ch = work.tile([DH, S], bf16, name=f"ch{pr}", tag="ch", bufs=2)
                nc.vector.tensor_mul(ch, cr[hl][0:DH, :], rb[0:DH, :])
                dma.dma_start(out=ctxn[pr][DH:2 * DH, :], in_=ch)

        def attention_pair(pr, filler, ctx_lag=2, dma=None):
            """Attention for head pair pr; `filler` is a list of zero-arg
            callables emitting tensor-engine bursts, interleaved one per key
            block to keep the in-order tensor queue busy while ScalarE exps.
            ctx matmuls lag the scores stream by ctx_lag (jb, hl) units."""
            cr = {}
            for hl in range(2):
                cr[hl] = psum.tile(
                    [DH + 1, S], f32, name=f"cr{pr}_{hl}", tag="cr", bufs=2
                )
            ctx_queue = []

            def emit_ctx(jb, hl, e):
                h = 2 * pr + hl
                for ic in range(2):
                    nc.tensor.matmul(
                        cr[hl][:, ic * 512:(ic + 1) * 512],
                        lhsT=vh_sb[jb][:, h, :],
                        rhs=e[:, ic * 512:(ic + 1) * 512],
                        start=(jb == 0),
                        stop=(jb == NJB - 1),
                    )

            for jb in range(NJB):
                # prefetch exp(bias) tiles for both heads of this key block
                ebs = []
                for hl in range(2):
                    h = 2 * pr + hl
                    eb = work.tile([P, S], bf16, name=f"eb{h}_{jb}", tag="eb", bufs=4)
                    nc.sync.dma_start(out=eb, in_=expb[h, jb * P:(jb + 1) * P, :])
                    ebs.append(eb)
                # scores: alternate PE row groups so hl0/hl1 run concurrently
                s_ps = [
                    psum.tile([P, S], f32, name=f"s{2 * pr + hl}_{jb}", tag="mm")
                    for hl in range(2)
                ]
                for ic in range(2):
                    for hl in range(2):
                        nc.tensor.matmul(
                            s_ps[hl][:, ic * 512:(ic + 1) * 512],
                            lhsT=khT[pr][hl * DH:(hl + 1) * DH, jb * P:(jb + 1) * P],
                            rhs=qhT[pr][hl * DH:(hl + 1) * DH, ic * 512:(ic + 1) * 512],
                            start=True,
                            stop=True,
                        )
                for hl in range(2):
                    h = 2 * pr + hl
                    es = work.tile([P, S], bf16, name=f"es{h}_{jb}", tag="es", bufs=3)
                    nc.scalar.activation(es, s_ps[hl], AF.Exp)
                    e = work.tile([P, S], bf16, name=f"e{h}_{jb}", tag="e", bufs=6)
                    nc.vector.tensor_mul(e, es, ebs[hl])
                    ctx_queue.append((jb, hl, e))
                    if len(ctx_queue) > ctx_lag:
                        emit_ctx(*ctx_queue.pop(0))
                if filler:
                    filler.pop(0)()
            while ctx_queue:
                emit_ctx(*ctx_queue.pop(0))
            return [
                (lambda hl=hl, dma=dma: normalize_head(pr, hl, cr, dma))
                for hl in range(2)
            ]

        # ---- schedule ----
        # pr0 q/k projection, then v (attention pr0 needs vh for ctx)
        for tname in ("q", "k"):
            for ic in range(2):
                qk_half_burst(0, tname, ic)
        for jb in range(NJB):
            v_burst(jb)
        # wo + remaining expb stream in DMA order behind the attention loads
        for pr in range(NPAIR):
            nc.sync.dma_start(
                out=wo_sb[:, pr, :],
                in_=wo.rearrange("(pr p) n -> p pr n", p=P)[:, pr, :],
            )

        def qk_fillers(pr):
            return [
                (lambda pr=pr, t=t, ic=ic: qk_half_burst(pr, t, ic))
                for t in ("q", "k")
                for ic in range(2)
            ]

        norm = attention_pair(0, qk_fillers(1), ctx_lag=3)
        for fn in norm:
            fn()
        norm = attention_pair(1, qk_fillers(2))
        for fn in norm:
            fn()
        norm = attention_pair(2, qk_fillers(3))
        for fn in norm:
            fn()
        norm = attention_pair(3, [], dma=nc.sync)

        # ---- output projection ----
        # pr0-2 contributions first: they run while pr3 normalizes.
        def outproj(ib, prs, start, stop, drain):
            yp = outproj.tiles.get(ib)
            if yp is None:
                yp = psum.tile([P, HID], f32, name=f"yp{ib}", tag="mm")
                outproj.tiles[ib] = yp
            for pr in prs:
                for cc in range(2):
                    nc.tensor.matmul(
                        yp[:, cc * 512:(cc + 1) * 512],
                        lhsT=ctxn[pr][:, ib * P:(ib + 1) * P],
                        rhs=wo_sb[:, pr, cc * 512:(cc + 1) * 512],
                        start=start and (pr == prs[0]),
                        stop=stop and (pr == prs[-1]),
                    )
            if drain:
                y_sb = outp.tile([P, HID], f32, name=f"y{ib}", tag="y")
                nc.scalar.copy(y_sb, yp)
                nc.sync.dma_start(out=out[ib * P:(ib + 1) * P, :], in_=y_sb)
                del outproj.tiles[ib]

        outproj.tiles = {}
        outproj(0, [0, 1, 2], start=True, stop=False, drain=False)
        outproj(1, [0, 1, 2], start=True, stop=False, drain=False)
        for fn in norm:
            fn()
        outproj(0, [3], start=False, stop=True, drain=True)
        for ib in range(2, NIB):
            outproj(ib, [0, 1, 2], start=True, stop=False, drain=False)
            outproj(ib - 1, [3], start=False, stop=True, drain=True)
        outproj(NIB - 1, [3], start=False, stop=True, drain=True)

    nc.compile()
    return nc


def _get_nc():
    global _CACHED_NC
    if _CACHED_NC is None:
        _CACHED_NC = _build_nc()
    return _CACHED_NC


def make_in_maps(q, k, v, attn_bias, Wq, Wk, Wv, Wo, bq, bk, bv, bo):
    scale = DH ** (-0.5)
    in_maps = []
    for core in range(8):
        b, g = divmod(core, 2)
        gs = slice(g * GCOL, (g + 1) * GCOL)
        in_maps.append({
            "qT": np.ascontiguousarray(q[b].T).astype(BF16),
            "kT": np.ascontiguousarray(k[b].T).astype(BF16),
            "vT": np.ascontiguousarray(v[b].T).astype(BF16),
            "wq": (Wq[:, gs] * scale).astype(BF16),
            "wk": np.ascontiguousarray(Wk[:, gs]).astype(BF16),
            "wv": np.ascontiguousarray(Wv[:, gs]).astype(BF16),
            "wo": np.ascontiguousarray(Wo[gs, :]).astype(BF16),
            "bq": (bq[gs] * scale).astype(np.float32),
            "bk": np.ascontiguousarray(bk[gs]).astype(np.float32),
            "bv": np.ascontiguousarray(bv[gs]).astype(BF16),
            "expb": np.exp(
                attn_bias[b, g * 8:(g + 1) * 8].transpose(0, 2, 1)
            ).astype(BF16),
        })
    return in_maps


def kernel(q, k, v, attn_bias, Wq, Wk, Wv, Wo, bq, bk, bv, bo, _trace=False):
    from concourse.bass_utils import run_bass_kernel_spmd

    args = [np.asarray(x, dtype=np.float32) for x in
            (q, k, v, attn_bias, Wq, Wk, Wv, Wo, bq, bk, bv, bo)]
    q, k, v, attn_bias, Wq, Wk, Wv, Wo, bq, bk, bv, bo = args
    nc = _get_nc()
    in_maps = make_in_maps(q, k, v, attn_bias, Wq, Wk, Wv, Wo, bq, bk, bv, bo)
    res = run_bass_kernel_spmd(nc, in_maps, core_ids=list(range(8)), trace=_trace)
    y = np.zeros((4, S, HID), np.float32)
    for core in range(8):
        y[core // 2] += res.results[core]["out"]
    y += bo
    if _trace:
        kernel.last_results = res
    return y


# revision 7
# speedup vs baseline: 1.1972x; 1.1972x over previous
"""Multi-head attention (B=4, S=1024, H=1024, heads=16) on 8 trn2 NeuronCores.

Sharding: data-parallel over batch (4) x tensor-parallel over head-groups (2).
Core c handles batch c//2, heads [8*(c%2), 8*(c%2)+8).

v2 schedule: the tensor queue is kept dense by interleaving projection
matmul bursts into the previous head-pair's attention loop, scores matmuls
alternate PE row-groups (hl0 rows 0-63 / hl1 rows 64-127) so they run
concurrently via tile_position, and the output projection accumulates
pr0-2 before pr3 so pr3's softmax-normalize hides under real matmuls.

Per-core math (all matmuls bf16 with fp32 PSUM accumulation):
  - projections: qh_T/kh_T in [d, i] layout (head dim on partitions), vh in
    [j, hd] layout augmented with a ones column per head (softmax
    denominator comes for free from the ctx matmul).
  - scores computed transposed (keys on partitions): s_T = khT^T @ qhT,
    exp on ScalarE, multiplied by host-precomputed exp(attn_bias)^T.
  - normalize with reciprocal of the denominator row + DMA partition
    broadcast, output projection with row-parallel Wo; host adds the two
    partial results + bo.

Scale (1/8) is folded into Wq/bq on the host. Softmax max-subtraction is
skipped: scores+bias are within +-8 so exp is well-conditioned in fp32.
"""

import numpy as np
import ml_dtypes

BF16 = ml_dtypes.bfloat16

S = 1024
HID = 1024
GCOL = 512  # hidden cols per core (8 heads * 64)
DH = 64
P = 128
NPAIR = 4  # head pairs per core
NJB = 8  # key blocks of 128
NCB = 8  # contraction blocks of 128
NIB = 8  # query blocks of 128

_CACHED_NC = None


def _build_nc():
    import concourse.bass as bass
    import concourse.mybir as mybir
    import concourse.tile as tile
    from concourse import bacc
    from contextlib import ExitStack

    f32 = mybir.dt.float32
    bf16 = mybir.dt.bfloat16
    AF = mybir.ActivationFunctionType

    nc = bacc.Bacc(
        "TRN2",
        target_bir_lowering=False,
        debug=False,
        enable_asserts=False,
        num_devices=8,
    )

    qT = nc.dram_tensor("qT", [HID, S], bf16, kind="ExternalInput").ap()
    kT = nc.dram_tensor("kT", [HID, S], bf16, kind="ExternalInput").ap()
    vT = nc.dram_tensor("vT", [HID, S], bf16, kind="ExternalInput").ap()
    wq = nc.dram_tensor("wq", [HID, GCOL], bf16, kind="ExternalInput").ap()
    wk = nc.dram_tensor("wk", [HID, GCOL], bf16, kind="ExternalInput").ap()
    wv = nc.dram_tensor("wv", [HID, GCOL], bf16, kind="ExternalInput").ap()
    wo = nc.dram_tensor("wo", [GCOL, HID], bf16, kind="ExternalInput").ap()
    bq = nc.dram_tensor("bq", [GCOL], f32, kind="ExternalInput").ap()
    bk = nc.dram_tensor("bk", [GCOL], f32, kind="ExternalInput").ap()
    bv = nc.dram_tensor("bv", [GCOL], bf16, kind="ExternalInput").ap()
    expb = nc.dram_tensor("expb", [8, S, S], bf16, kind="ExternalInput").ap()
    out = nc.dram_tensor("out", [S, HID], f32, kind="ExternalOutput").ap()

    with tile.TileContext(nc) as tc, ExitStack() as ctx:
        const = ctx.enter_context(tc.tile_pool(name="const", bufs=1))
        inT = ctx.enter_context(tc.tile_pool(name="inT", bufs=24))
        proj = ctx.enter_context(tc.tile_pool(name="proj", bufs=1))
        work = ctx.enter_context(tc.tile_pool(name="work", bufs=6))
        outp = ctx.enter_context(tc.tile_pool(name="outp", bufs=2))
        psum = ctx.enter_context(tc.tile_pool(name="psum", bufs=2, space="PSUM"))

        # ---- constants / weights (DMA priority: q, k first — scores path) ----
        wq_sb = const.tile([P, NCB, GCOL], bf16, tag="wq")
        wk_sb = const.tile([P, NCB, GCOL], bf16, tag="wk")
        wv_sb = const.tile([P, NCB, GCOL], bf16, tag="wv")
        wo_sb = const.tile([P, NPAIR, HID], bf16, tag="wo")
        wq_r = wq.rearrange("(cb p) n -> p cb n", p=P)
        wk_r = wk.rearrange("(cb p) n -> p cb n", p=P)
        wv_r = wv.rearrange("(cb p) n -> p cb n", p=P)
        bq_sb = const.tile([P, NPAIR], f32, tag="bq")
        bk_sb = const.tile([P, NPAIR], f32, tag="bk")
        bv_sb = const.tile([1, GCOL], bf16, tag="bv")
        ones_k1 = const.tile([1, P], bf16, tag="ones_k1")
        nc.vector.memset(ones_k1, 1.0)
        # pre-warm the Exp activation table before the attention phase
        warm = const.tile([1, 16], bf16, tag="warm")
        nc.vector.memset(warm, 0.0)
        nc.scalar.activation(warm, warm, AF.Exp)

        qhT = [proj.tile([P, S], bf16, name=f"qhT{i}", tag=f"qhT{i}") for i in range(NPAIR)]
        khT = [proj.tile([P, S], bf16, name=f"khT{i}", tag=f"khT{i}") for i in range(NPAIR)]
        # vh_sb[jb]: [j in block, head, 65] where col 64 is ones (denominator trick)
        vh_sb = [proj.tile([P, 8, DH + 1], bf16, name=f"vh{i}", tag=f"vh{i}") for i in range(NJB)]
        ctxn = [proj.tile([P, S], bf16, name=f"ctxn{i}", tag=f"ctxn{i}") for i in range(NPAIR)]

        # input DMAs: q/k inputs + weights first so pr0 scores start early
        qk_tiles = {}
        nc.sync.dma_start(out=bq_sb, in_=bq.rearrange("(pr p) -> p pr", p=P))
        nc.sync.dma_start(out=bk_sb, in_=bk.rearrange("(pr p) -> p pr", p=P))
        for tname, src, w_r, w_sb in (("q", qT, wq_r, wq_sb), ("k", kT, wk_r, wk_sb)):
            tl = []
            for cb in range(NCB):
                nc.sync.dma_start(out=w_sb[:, cb, :], in_=w_r[:, cb, :])
                t = inT.tile([P, S], bf16, name=f"{tname}T{cb}", tag="inT")
                nc.sync.dma_start(out=t, in_=src[cb * P:(cb + 1) * P, :])
                tl.append(t)
            qk_tiles[tname] = tl
        vtiles = []
        nc.sync.dma_start(out=bv_sb, in_=bv.rearrange("(a n) -> a n", a=1))
        for cb in range(NCB):
            nc.sync.dma_start(out=wv_sb[:, cb, :], in_=wv_r[:, cb, :])
            t = inT.tile([P, S], bf16, name=f"vT{cb}", tag="inT")
            nc.sync.dma_start(out=t, in_=vT[cb * P:(cb + 1) * P, :])
            vtiles.append(t)

        def qk_half_burst(pr, tname, ic):
            """8 matmuls accumulating one [128, 512] half of q/k projection."""
            w_sb, b_sb, dst = (
                (wq_sb, bq_sb, qhT) if tname == "q" else (wk_sb, bk_sb, khT)
            )
            ps = psum.tile([P, S], f32, name=f"{tname}p{pr}_{ic}", tag="mm")
            for cb in range(NCB):
                nc.tensor.matmul(
                    ps[:, 0:512],
                    lhsT=w_sb[:, cb, pr * P:(pr + 1) * P],
                    rhs=qk_tiles[tname][cb][:, ic * 512:(ic + 1) * 512],
                    start=(cb == 0),
                    stop=(cb == NCB - 1),
                )
            nc.vector.tensor_scalar_add(
                dst[pr][:, ic * 512:(ic + 1) * 512], ps[:, 0:512], b_sb[:, pr:pr + 1]
            )

        def v_burst(jb):
            ps = psum.tile([P, S], f32, name=f"vp{jb}", tag="mm")
            for cb in range(NCB):
                nc.tensor.matmul(
                    ps[:, 0:GCOL],
                    lhsT=vtiles[cb][:, jb * P:(jb + 1) * P],
                    rhs=wv_sb[:, cb, :],
                    start=(cb == 0),
                    stop=False,
                )
            nc.tensor.matmul(ps[:, 0:GCOL], lhsT=ones_k1, rhs=bv_sb, start=False, stop=True)
            nc.vector.tensor_copy(
                out=vh_sb[jb][:, :, 0:DH],
                in_=ps[:, 0:GCOL].rearrange("p (h d) -> p h d", d=DH),
            )
            nc.vector.memset(vh_sb[jb][:, :, DH:DH + 1], 1.0)

        def normalize_head(pr, hl, cr, dma=None):
            """Normalize ctx rows by 1/r and place into ctxn[pr].

            One PSUM->SBUF copy frees the cr bank immediately; the rest of the
            chain (partition shift + approx reciprocal + DMA broadcast + mul)
            runs from SBUF off the attention critical path."""
            dma = dma or nc.gpsimd
            cs = work.tile([DH + 1, S], bf16, name=f"cs{pr}{hl}", tag="cs", bufs=2)
            nc.vector.tensor_copy(cs, cr[hl])
            rb = work.tile([P, S], f32, name=f"rb{pr}{hl}", tag="rb", bufs=2)
            nc.vector.tensor_copy(rb[DH:DH + 1, :], cr[hl][DH:DH + 1, :])
            dma.dma_start(out=rb[0:1, :], in_=rb[DH:DH + 1, :])
            nc.vector.reciprocal_approx_fast(rb[0:1, :], rb[0:1, :])
            row = rb[0:1, :]
            row8 = bass.AP(
                tensor=row.tensor,
                offset=row.offset,
                ap=[list(row.ap[0]), [0, 7]] + [list(d) for d in row.ap[1:]],
            )
            dma.dma_start(out=rb[1:8, :], in_=row8)
            blk = rb[0:8, :]
            blk_rep = bass.AP(
                tensor=blk.tensor,
                offset=blk.offset,
                ap=[list(blk.ap[0]), [0, 7]] + [list(d) for d in blk.ap[1:]],
            )
            dma.dma_start(out=rb[8:DH, :], in_=blk_rep)
            if hl == 0:
                nc.vector.tensor_mul(ctxn[pr][0:DH, :], cs[0:DH, :], rb[0:DH, :])
            else:
                ch = work.tile([DH, S], bf16, name=f"ch{pr}", tag="ch", bufs=2)
                nc.vector.tensor_mul(ch, cs[0:DH, :], rb[0:DH, :])
                dma.dma_start(out=ctxn[pr][DH:2 * DH, :], in_=ch)

        def attention_pair(pr, filler, ctx_lag=2, dma=None):
            """Attention for head pair pr; `filler` is a list of zero-arg
            callables emitting tensor-engine bursts, interleaved one per key
            block to keep the in-order tensor queue busy while ScalarE exps.
            ctx matmuls lag the scores stream by ctx_lag (jb, hl) units."""
            cr = {}
            for hl in range(2):
                cr[hl] = psum.tile(
                    [DH + 1, S], f32, name=f"cr{pr}_{hl}", tag="cr", bufs=2
                )
            ctx_queue = []

            def emit_ctx(jb, hl, e):
                h = 2 * pr + hl
                for ic in range(2):
                    nc.tensor.matmul(
                        cr[hl][:, ic * 512:(ic + 1) * 512],
                        lhsT=vh_sb[jb][:, h, :],
                        rhs=e[:, ic * 512:(ic + 1) * 512],
                        start=(jb == 0),
                        stop=(jb == NJB - 1),
                    )

            for jb in range(NJB):
                # filler burst first: its DVE drain lands ahead of this slot's
                # attention muls in the in-order vector queue
                if filler:
                    filler.pop(0)()
                # prefetch exp(bias) tiles for both heads of this key block
                ebs = []
                for hl in range(2):
                    h = 2 * pr + hl
                    eb = work.tile([P, S], bf16, name=f"eb{h}_{jb}", tag="eb", bufs=4)
                    nc.sync.dma_start(out=eb, in_=expb[h, jb * P:(jb + 1) * P, :])
                    ebs.append(eb)
                # scores: alternate PE row groups so hl0/hl1 run concurrently
                s_ps = [
                    psum.tile([P, S], f32, name=f"s{2 * pr + hl}_{jb}", tag="mm")
                    for hl in range(2)
                ]
                for ic in range(2):
                    for hl in range(2):
                        nc.tensor.matmul(
                            s_ps[hl][:, ic * 512:(ic + 1) * 512],
                            lhsT=khT[pr][hl * DH:(hl + 1) * DH, jb * P:(jb + 1) * P],
                            rhs=qhT[pr][hl * DH:(hl + 1) * DH, ic * 512:(ic + 1) * 512],
                            start=True,
                            stop=True,
                        )
                for hl in range(2):
                    h = 2 * pr + hl
                    es = work.tile([P, S], bf16, name=f"es{h}_{jb}", tag="es", bufs=3)
                    nc.scalar.activation(es, s_ps[hl], AF.Exp)
                    e = work.tile([P, S], bf16, name=f"e{h}_{jb}", tag="e", bufs=6)
                    nc.vector.tensor_mul(e, es, ebs[hl])
                    ctx_queue.append((jb, hl, e))
                    if len(ctx_queue) > ctx_lag:
                        emit_ctx(*ctx_queue.pop(0))
            while ctx_queue:
                emit_ctx(*ctx_queue.pop(0))
            return [
                (lambda hl=hl, dma=dma: normalize_head(pr, hl, cr, dma))
                for hl in range(2)
            ]

        # ---- schedule ----
        # pr0 q/k projection, then v (attention pr0 needs vh for ctx)
        for tname in ("q", "k"):
            for ic in range(2):
                qk_half_burst(0, tname, ic)
        for jb in range(NJB):
            v_burst(jb)
        # wo + remaining expb stream in DMA order behind the attention loads
        for pr in range(NPAIR):
            nc.sync.dma_start(
                out=wo_sb[:, pr, :],
                in_=wo.rearrange("(pr p) n -> p pr n", p=P)[:, pr, :],
            )

        def qk_fillers(pr):
            return [
                (lambda pr=pr, t=t, ic=ic: qk_half_burst(pr, t, ic))
                for t in ("q", "k")
                for ic in range(2)
            ]

        norm = attention_pair(0, qk_fillers(1), ctx_lag=3)
        for fn in norm:
            fn()
        norm = attention_pair(1, qk_fillers(2))
        for fn in norm:
            fn()
        norm = attention_pair(2, qk_fillers(3))
        for fn in norm:
            fn()
        norm = attention_pair(3, [], dma=nc.sync)

        # ---- output projection ----
        # pr0-2 contributions first: they run while pr3 normalizes.
        def outproj(ib, prs, start, stop, drain):
            yp = outproj.tiles.get(ib)
            if yp is None:
                yp = psum.tile([P, HID], f32, name=f"yp{ib}", tag="mm")
                outproj.tiles[ib] = yp
            for pr in prs:
                for cc in range(2):
                    nc.tensor.matmul(
                        yp[:, cc * 512:(cc + 1) * 512],
                        lhsT=ctxn[pr][:, ib * P:(ib + 1) * P],
                        rhs=wo_sb[:, pr, cc * 512:(cc + 1) * 512],
                        start=start and (pr == prs[0]),
                        stop=stop and (pr == prs[-1]),
                    )
            if drain:
                y_sb = outp.tile([P, HID], f32, name=f"y{ib}", tag="y")
                nc.scalar.copy(y_sb, yp)
                nc.sync.dma_start(out=out[ib * P:(ib + 1) * P, :], in_=y_sb)
                del outproj.tiles[ib]

        outproj.tiles = {}
        outproj(0, [0, 1, 2], start=True, stop=False, drain=False)
        outproj(1, [0, 1, 2], start=True, stop=False, drain=False)
        for fn in norm:
            fn()
        outproj(0, [3], start=False, stop=True, drain=True)
        for ib in range(2, NIB):
            outproj(ib, [0, 1, 2], start=True, stop=False, drain=False)
            outproj(ib - 1, [3], start=False, stop=True, drain=True)
        outproj(NIB - 1, [3], start=False, stop=True, drain=True)

    nc.compile()
    return nc


def _get_nc():
    global _CACHED_NC
    if _CACHED_NC is None:
        _CACHED_NC = _build_nc()
    return _CACHED_NC


def make_in_maps(q, k, v, attn_bias, Wq, Wk, Wv, Wo, bq, bk, bv, bo):
    scale = DH ** (-0.5)
    in_maps = []
    for core in range(8):
        b, g = divmod(core, 2)
        gs = slice(g * GCOL, (g + 1) * GCOL)
        in_maps.append({
            "qT": np.ascontiguousarray(q[b].T).astype(BF16),
            "kT": np.ascontiguousarray(k[b].T).astype(BF16),
            "vT": np.ascontiguousarray(v[b].T).astype(BF16),
            "wq": (Wq[:, gs] * scale).astype(BF16),
            "wk": np.ascontiguousarray(Wk[:, gs]).astype(BF16),
            "wv": np.ascontiguousarray(Wv[:, gs]).astype(BF16),
            "wo": np.ascontiguousarray(Wo[gs, :]).astype(BF16),
            "bq": (bq[gs] * scale).astype(np.float32),
            "bk": np.ascontiguousarray(bk[gs]).astype(np.float32),
            "bv": np.ascontiguousarray(bv[gs]).astype(BF16),
            "expb": np.exp(
                attn_bias[b, g * 8:(g + 1) * 8].transpose(0, 2, 1)
            ).astype(BF16),
        })
    return in_maps


def kernel(q, k, v, attn_bias, Wq, Wk, Wv, Wo, bq, bk, bv, bo, _trace=False):
    from concourse.bass_utils import run_bass_kernel_spmd

    args = [np.asarray(x, dtype=np.float32) for x in
            (q, k, v, attn_bias, Wq, Wk, Wv, Wo, bq, bk, bv, bo)]
    q, k, v, attn_bias, Wq, Wk, Wv, Wo, bq, bk, bv, bo = args
    nc = _get_nc()
    in_maps = make_in_maps(q, k, v, attn_bias, Wq, Wk, Wv, Wo, bq, bk, bv, bo)
    res = run_bass_kernel_spmd(nc, in_maps, core_ids=list(range(8)), trace=_trace)
    y = np.zeros((4, S, HID), np.float32)
    for core in range(8):
        y[core // 2] += res.results[core]["out"]
    y += bo
    if _trace:
        kernel.last_results = res
    return y
